# revision 16
# baseline (speedup 1.0000x reference)
"""Trainium2 Bass kernel: CustomTransformerEncoderLayer, 8-core SPMD.

Sharding: core c handles batch b=c//4 and query tokens [qq*512, qq*512+512)
with qq=c%4.  Keys/values span the whole batch, so each core computes K/V
for all 4 quarters of its batch (replicated across the 4 cores of a batch;
collectives are avoided on purpose) and everything else only for its own
512-token query slice.

Dtype strategy: all projection/FFN GEMMs run in fp8-e4m3 with
perf_mode=DoubleRow (2 contraction rows per PE cell, 2x rate); the
attention-score and attn@V GEMMs and the LN stats run in bf16 (full PE
rate); PSUM accumulation is always fp32; LN statistics and softmax
normalization are fp32.  The residual path keeps a bf16 copy of src.
v_b is folded into o_b on the host (normalized probs sum to 1), and
alpha_attn / alpha_ff / 1/sqrt(DH) are folded into weights/biases.

Attention keeps all of K^T, V resident in SBUF (bf16) and accumulates
attn@V over all 16 key chunks per head-pair directly in PSUM, so the
softmax needs no cross-quarter accumulators: exp is unnormalized, the row
sums come from a ones-column appended to V, and normalization is applied
to the tiny attention output.  The probability mask is a bf16 multiply on
DVE against the exp output (exp writes bf16)."""

from contextlib import ExitStack

import numpy as np
import ml_dtypes

import concourse.bass as bass
import concourse.bacc as bacc
from concourse import mybir
from concourse.tile import TileContext
from concourse.bass_utils import run_bass_kernel_spmd
from concourse.masks import make_identity

P = 128
B, T, D, H, DH, F = 2, 2048, 1024, 16, 64, 4096
TQ = 512          # query tokens per core
EC = D // P       # 8 feature chunks of 128
CC = D // 256     # 4 DoubleRow contraction chunks of 256
FC = F // P       # 32 ff chunks
FCC = F // 256    # 16 DoubleRow chunks of d_ff
NQ = 4            # key quarters per batch
NCH = T // P      # 16 key chunks of 128
NCORES = 8
EPS_LN = 1e-5

f32 = mybir.dt.float32
bf16 = mybir.dt.bfloat16
fp8 = mybir.dt.float8e4
np_fp8 = ml_dtypes.float8_e4m3
np_bf16 = ml_dtypes.bfloat16
ADD = mybir.AluOpType.add
MUL = mybir.AluOpType.mult
SUB = mybir.AluOpType.subtract
AF = mybir.ActivationFunctionType
DR = mybir.MatmulPerfMode.DoubleRow

_NC_CACHE = None


def _dram(nc, name, shape, dtype, out=False):
    return nc.dram_tensor(name, list(shape), dtype,
                          kind="ExternalOutput" if out else "ExternalInput")


def _build_nc(repeat=1, dups=1):
    nc = bacc.Bacc("TRN2", target_bir_lowering=False, debug=False)

    srcb = _dram(nc, "srcb", (T, D), bf16)            # own batch, natural order
    qw = _dram(nc, "qw", (EC, P, CC * 2 * P), fp8)    # DR lhsT packs
    kw = _dram(nc, "kw", (P, EC * CC * 2 * P), fp8)   # partition-major resident
    ow = _dram(nc, "ow", (EC, P, CC * 2 * P), fp8)
    vw = _dram(nc, "vw", (P, CC * 2 * D), fp8)        # partition-major resident
    l1w = _dram(nc, "l1w", (FC, P, CC * 2 * P), fp8)
    l2w = _dram(nc, "l2w", (EC, P, FCC * 2 * P), fp8)
    mask = _dram(nc, "mask", (P, NCH, TQ), bf16)      # keep-mask [s-part, ch, t]
    qb = _dram(nc, "qb", (P, EC), f32)                # (q_b/sqrt(DH)) cols
    kb = _dram(nc, "kb", (P, EC), f32)
    ob = _dram(nc, "ob", (P, EC), f32)                # alpha*(o_b + o_w@v_b)
    l1b = _dram(nc, "l1b", (P, FC), f32)
    l2b = _dram(nc, "l2b", (P, EC), f32)              # alpha_ff folded
    n1s = _dram(nc, "n1s", (P, EC), f32)
    n1b = _dram(nc, "n1b", (P, EC), f32)
    n2s = _dram(nc, "n2s", (P, EC), f32)
    n2b = _dram(nc, "n2b", (P, EC), f32)
    al_a = _dram(nc, "al_a", (P, 1), f32)             # alpha_attn column
    al_f = _dram(nc, "al_f", (P, 1), f32)             # alpha_ff column
    out = _dram(nc, "out", (D, TQ), f32, out=True)    # transposed output

    with TileContext(nc) as tc, ExitStack() as octx:
        consts = octx.enter_context(tc.tile_pool(name="consts", bufs=1))
        persist = octx.enter_context(tc.tile_pool(name="persist", bufs=1))

        ident = consts.tile([P, P], bf16)
        make_identity(nc, ident)
        ones_col = consts.tile([P, 1], bf16)
        nc.vector.memset(ones_col, 1.0)
        eps_t = consts.tile([1, 1], f32)
        nc.vector.memset(eps_t, EPS_LN)

        def load_small(ext, cols, tag):
            t = consts.tile([P, cols], f32, tag=tag)
            nc.sync.dma_start(out=t, in_=ext.ap())
            return t

        qb_t = load_small(qb, EC, "qb")
        kb_t = load_small(kb, EC, "kb")
        ob_t = load_small(ob, EC, "ob")
        l1b_t = load_small(l1b, FC, "l1b")
        l2b_t = load_small(l2b, EC, "l2b")
        n1s_t = load_small(n1s, EC, "n1s")
        n1b_t = load_small(n1b, EC, "n1b")
        n2s_t = load_small(n2s, EC, "n2s")
        n2b_t = load_small(n2b, EC, "n2b")
        ala_t = load_small(al_a, 1, "ala")
        alf_t = load_small(al_f, 1, "alf")

        # K^T / V resident for the whole batch; ones column of V set once
        # (attention only ever writes columns 0:DH).
        ktq_all = persist.tile([P, EC, T], bf16, tag="ktq")
        vq_all = persist.tile([P, NCH, H, DH + 1], bf16, tag="vq")
        nc.vector.memset(vq_all[:, :, :, DH:DH + 1], 1.0)
        qT = persist.tile([P, EC, TQ], bf16, tag="qT")
        srcT_own = persist.tile([P, EC, TQ], bf16, tag="srcTown")
        attnT8 = persist.tile([P, EC, TQ], fp8, tag="attnT8")
        xT = persist.tile([P, EC, TQ], bf16, tag="xT")
        xT8 = persist.tile([P, EC, TQ], fp8, tag="xT8")

        src4 = srcb.ap().rearrange("(ch p) e -> ch p e", p=P)  # 16 chunks

        rep_ctx = ExitStack()
        if repeat > 1:
            rep_ctx.enter_context(tc.For_i(0, repeat, 1))

        for _dup in range(dups):
            # ============ phase 1: transposes + K/V (all quarters) + Q ======
            with ExitStack() as ctx:
                attp = ctx.enter_context(tc.tile_pool(name="attp", bufs=1))
                mask_t = attp.tile([P, NCH, TQ], bf16, tag="mask")

                p1 = ExitStack()
                srcio = p1.enter_context(tc.tile_pool(name="srcio", bufs=2))
                s8p = p1.enter_context(tc.tile_pool(name="s8p", bufs=2))
                wres = p1.enter_context(tc.tile_pool(name="wres", bufs=1))
                wq = p1.enter_context(tc.tile_pool(name="wq", bufs=2))
                pst = p1.enter_context(
                    tc.tile_pool(name="pst", bufs=2, space="PSUM"))
                psmm = p1.enter_context(
                    tc.tile_pool(name="psmm", bufs=3, space="PSUM"))

                kw_sb = wres.tile([P, EC, CC, 2, P], fp8, tag="kw")
                vw_sb = wres.tile([P, CC, 2, D], fp8, tag="vw")

                for q in range(NQ):
                    srcT8 = s8p.tile([P, EC, TQ], fp8, tag="srcT8")
                    own = q == NQ - 1  # own quarter rotated last (host prep)
                    for sc in range(4):
                        s_tile = srcio.tile([P, D], bf16, tag="srcin")
                        nc.sync.dma_start(out=s_tile, in_=src4[q * 4 + sc])
                        for jh in range(2):
                            pst4 = pst.tile([P, 4, P], bf16, tag="tps")
                            for i in range(4):
                                j = 4 * jh + i
                                nc.tensor.transpose(
                                    pst4[:, i, :],
                                    s_tile[:, j * P:(j + 1) * P], ident)
                            nc.vector.tensor_copy(
                                out=srcT8[:, 4 * jh:4 * jh + 4,
                                          sc * P:(sc + 1) * P],
                                in_=pst4)
                            if own:
                                nc.vector.tensor_copy(
                                    out=srcT_own[:, 4 * jh:4 * jh + 4,
                                                 sc * P:(sc + 1) * P],
                                    in_=pst4)
                    if q == 0:
                        # emitted after q0's transposes so the first src
                        # chunk DMA isn't queued behind 2MB of weights
                        nc.sync.dma_start(
                            out=kw_sb,
                            in_=kw.ap().rearrange(
                                "p (e c j m) -> p e c j m", e=EC, c=CC, j=2))
                        nc.sync.dma_start(
                            out=vw_sb,
                            in_=vw.ap().rearrange(
                                "p (c j d) -> p c j d", c=CC, j=2))
                    # K projection for this quarter (fp8 DoubleRow)
                    for dp in range(EC):
                        ps = psmm.tile([P, TQ], f32, tag="mm")
                        for cc in range(CC):
                            nc.tensor.matmul(
                                ps, kw_sb[:, dp, cc, :, :],
                                srcT8[:, 2 * cc:2 * cc + 2, :],
                                start=(cc == 0), stop=(cc == CC - 1),
                                perf_mode=DR)
                        nc.scalar.activation(
                            ktq_all[:, dp, q * TQ:(q + 1) * TQ], ps,
                            AF.Identity, bias=kb_t[:, dp:dp + 1])
                    # V projection for this quarter (fp8 DoubleRow, src
                    # stationary, weights moving); v_b folded into ob.
                    for dn in range(2):
                        for sc in range(4):
                            ps = psmm.tile([P, TQ], f32, tag="mm")
                            for cc in range(CC):
                                nc.tensor.matmul(
                                    ps,
                                    srcT8[:, 2 * cc:2 * cc + 2,
                                          sc * P:(sc + 1) * P],
                                    vw_sb[:, cc, :, dn * TQ:(dn + 1) * TQ],
                                    start=(cc == 0), stop=(cc == CC - 1),
                                    perf_mode=DR)
                            if dn == 0:
                                nc.scalar.activation(
                                    vq_all[:, q * 4 + sc, 0:8, 0:DH],
                                    ps.rearrange("p (h x) -> p h x", x=DH),
                                    AF.Identity)
                            else:
                                nc.vector.tensor_copy(
                                    out=vq_all[:, q * 4 + sc, 8:16, 0:DH],
                                    in_=ps.rearrange("p (h x) -> p h x",
                                                     x=DH))
                    if own:
                        for dp in range(EC):
                            w_t = wq.tile([P, CC, 2, P], fp8, tag="qw")
                            nc.sync.dma_start(
                                out=w_t,
                                in_=qw.ap()[dp].rearrange(
                                    "p (c j m) -> p c j m", j=2, m=P))
                            ps = psmm.tile([P, TQ], f32, tag="mm")
                            for cc in range(CC):
                                nc.tensor.matmul(
                                    ps, w_t[:, cc, :, :],
                                    srcT8[:, 2 * cc:2 * cc + 2, :],
                                    start=(cc == 0), stop=(cc == CC - 1),
                                    perf_mode=DR)
                            nc.scalar.activation(
                                qT[:, dp, :], ps, AF.Identity,
                                bias=qb_t[:, dp:dp + 1])

                p1.close()

                # ============ phase 2: attention ============
                probs_pool = ctx.enter_context(
                    tc.tile_pool(name="probs", bufs=4))
                rbp = ctx.enter_context(tc.tile_pool(name="rbp", bufs=2))
                pssc = ctx.enter_context(
                    tc.tile_pool(name="pssc", bufs=2, space="PSUM"))
                psav = ctx.enter_context(
                    tc.tile_pool(name="psav", bufs=2, space="PSUM"))

                nc.sync.dma_start(out=mask_t, in_=mask.ap())

                def scores(pr, ch):
                    ps_sc = pssc.tile([P, 2, TQ], f32, tag="sc")
                    nc.tensor.matmul(
                        ps_sc[:, 0, :],
                        ktq_all[0:DH, pr, ch * P:(ch + 1) * P],
                        qT[0:DH, pr, :], start=True, stop=True)
                    nc.tensor.matmul(
                        ps_sc[:, 1, :],
                        ktq_all[DH:P, pr, ch * P:(ch + 1) * P],
                        qT[DH:P, pr, :], start=True, stop=True)
                    return ps_sc

                cur = scores(0, 0)
                for pr in range(EC):
                    psA = psav.tile([DH + 1, TQ], f32, tag="ava")
                    psB = psav.tile([DH + 1, TQ], f32, tag="avb")
                    for ch in range(NCH):
                        prb = probs_pool.tile([P, 2, TQ], bf16, tag="probs")
                        nc.scalar.activation(prb, cur, AF.Exp, scale=0.125)
                        meng = nc.gpsimd if ch % 4 == 3 else nc.vector
                        meng.tensor_tensor(
                            prb, prb,
                            mask_t[:, ch, None, :].to_broadcast((P, 2, TQ)),
                            MUL)
                        # emit next chunk's scores BEFORE attn@V so the PE
                        # stream never blocks on the exp/mask of this chunk
                        if ch + 1 < NCH:
                            cur = scores(pr, ch + 1)
                        elif pr + 1 < EC:
                            cur = scores(pr + 1, 0)
                        nc.tensor.matmul(
                            psA, vq_all[:, ch, 2 * pr, :], prb[:, 0, :],
                            start=(ch == 0), stop=(ch == NCH - 1))
                        nc.tensor.matmul(
                            psB, vq_all[:, ch, 2 * pr + 1, :], prb[:, 1, :],
                            start=(ch == 0), stop=(ch == NCH - 1))
                    # normalize -> attnT8 (head-pair pr = feature chunk pr).
                    # The sums rows sit at PSUM partition 64; DVE can't shift
                    # partitions, so stage them in SBUF at partition 64 and
                    # DMA both down to partition 0 in one transfer.
                    scr = rbp.tile([P, 2, TQ], f32, tag="scr")
                    nc.vector.tensor_copy(out=scr[DH:DH + 1, 0, :],
                                          in_=psA[DH:DH + 1, :])
                    nc.vector.tensor_copy(out=scr[DH:DH + 1, 1, :],
                                          in_=psB[DH:DH + 1, :])
                    rows = rbp.tile([1, 2, TQ], f32, tag="rows")
                    nc.sync.dma_start(out=rows, in_=scr[DH:DH + 1, :, :])
                    nc.vector.reciprocal(rows, rows)
                    tmp = rbp.tile([DH, 2, TQ], f32, tag="rb")
                    nc.gpsimd.partition_broadcast(tmp, rows)
                    nc.vector.tensor_tensor(
                        attnT8[0:DH, pr, :], psA[0:DH, :], tmp[:, 0, :], MUL)
                    nB8 = rbp.tile([DH, TQ], fp8, tag="nb8")
                    nc.vector.tensor_tensor(nB8, psB[0:DH, :], tmp[:, 1, :],
                                            MUL)
                    nc.sync.dma_start(out=attnT8[DH:P, pr, :], in_=nB8)

            def layernorm(ctx2, zT, s_t, b_t, sink):
                """zT [128, EC, 512] bf16; sink(ko, ap) consumes chunks."""
                lnp = ctx2.enter_context(tc.tile_pool(name="lnp", bufs=2))
                lns = ctx2.enter_context(tc.tile_pool(name="lns", bufs=1))
                psrow = ctx2.enter_context(
                    tc.tile_pool(name="psrow", bufs=1, space="PSUM"))
                ps_s = psrow.tile([1, TQ], f32, tag="sum")
                ps_q = psrow.tile([1, TQ], f32, tag="sumsq")
                for ko in range(EC):
                    sq = lnp.tile([P, TQ], bf16, tag="sq")
                    nc.vector.tensor_tensor(sq, zT[:, ko, :], zT[:, ko, :],
                                            MUL)
                    nc.tensor.matmul(ps_s, ones_col, zT[:, ko, :],
                                     start=(ko == 0), stop=(ko == EC - 1))
                    nc.tensor.matmul(ps_q, ones_col, sq,
                                     start=(ko == 0), stop=(ko == EC - 1))
                st = lns.tile([1, 4, TQ], f32, tag="stats")
                mu, var, a_row, b_row = (st[:, 0, :], st[:, 1, :],
                                         st[:, 2, :], st[:, 3, :])
                nc.vector.tensor_scalar_mul(mu, ps_s, 1.0 / D)
                nc.vector.tensor_scalar_mul(var, ps_q, 1.0 / D)
                nc.vector.tensor_tensor(a_row, mu, mu, MUL)
                nc.vector.tensor_tensor(var, var, a_row, SUB)
                nc.scalar.activation(var, var, AF.Sqrt, bias=eps_t)
                nc.vector.reciprocal(a_row, var)
                nc.vector.tensor_tensor(b_row, mu, a_row, MUL)
                nc.vector.tensor_scalar_mul(b_row, b_row, -1.0)
                ab = lns.tile([P, 2, TQ], f32, tag="ab")
                nc.gpsimd.partition_broadcast(ab[:, 0, :], a_row)
                nc.gpsimd.partition_broadcast(ab[:, 1, :], b_row)
                for ko in range(EC):
                    r = lnp.tile([P, TQ], f32, tag="res")
                    nc.vector.tensor_tensor(r, zT[:, ko, :], ab[:, 0, :], MUL)
                    nc.vector.tensor_tensor(r, r, ab[:, 1, :], ADD)
                    sink(ko, r)

            # ============ phase 3+4: out-proj + LN1 + FFN + LN2 ============
            with ExitStack() as ctx:
                wstr = ctx.enter_context(tc.tile_pool(name="wstr", bufs=2))
                zp = ctx.enter_context(tc.tile_pool(name="zp", bufs=1))
                hp = ctx.enter_context(tc.tile_pool(name="hp", bufs=1))
                psmm = ctx.enter_context(
                    tc.tile_pool(name="psmm2", bufs=3, space="PSUM"))

                l2w_sb = hp.tile([P, EC, FCC, 2, P], fp8, tag="l2w")
                nc.sync.dma_start(
                    out=l2w_sb,
                    in_=l2w.ap().rearrange("e p (c j m) -> p e c j m",
                                           j=2, m=P))

                zT = zp.tile([P, EC, TQ], bf16, tag="zT")
                for ep in range(EC):
                    w_t = wstr.tile([P, CC, 2, P], fp8, tag="ow")
                    nc.sync.dma_start(
                        out=w_t,
                        in_=ow.ap()[ep].rearrange(
                            "p (c j m) -> p c j m", j=2, m=P))
                    ps = psmm.tile([P, TQ], f32, tag="mm")
                    for cc in range(CC):
                        nc.tensor.matmul(
                            ps, w_t[:, cc, :, :],
                            attnT8[:, 2 * cc:2 * cc + 2, :],
                            start=(cc == 0), stop=(cc == CC - 1),
                            perf_mode=DR)
                    nc.scalar.activation(zT[:, ep, :], ps, AF.Identity,
                                         scale=ala_t[:, 0:1],
                                         bias=ob_t[:, ep:ep + 1])
                    nc.vector.tensor_tensor(zT[:, ep, :], zT[:, ep, :],
                                            srcT_own[:, ep, :], ADD)

                def to_xT(ko, r):
                    nc.scalar.activation(xT[:, ko, :], r, AF.Identity,
                                         scale=n1s_t[:, ko:ko + 1],
                                         bias=n1b_t[:, ko:ko + 1])
                    nc.gpsimd.tensor_copy(out=xT8[:, ko, :],
                                          in_=xT[:, ko, :])

                layernorm(ctx, zT, n1s_t, n1b_t, to_xT)

                hT8 = hp.tile([P, FC, TQ], fp8, tag="hT8")
                for fp in range(FC):
                    w_t = wstr.tile([P, CC, 2, P], fp8, tag="l1w")
                    nc.sync.dma_start(
                        out=w_t,
                        in_=l1w.ap()[fp].rearrange(
                            "p (c j m) -> p c j m", j=2, m=P))
                    ps = psmm.tile([P, TQ], f32, tag="mm")
                    for cc in range(CC):
                        nc.tensor.matmul(
                            ps, w_t[:, cc, :, :],
                            xT8[:, 2 * cc:2 * cc + 2, :],
                            start=(cc == 0), stop=(cc == CC - 1),
                            perf_mode=DR)
                    nc.scalar.activation(hT8[:, fp, :], ps, AF.Relu,
                                         bias=l1b_t[:, fp:fp + 1])

                z2T = zp.tile([P, EC, TQ], bf16, tag="z2T")
                for ep in range(EC):
                    ps = psmm.tile([P, TQ], f32, tag="mm")
                    for cc in range(FCC):
                        nc.tensor.matmul(
                            ps, l2w_sb[:, ep, cc, :, :],
                            hT8[:, 2 * cc:2 * cc + 2, :],
                            start=(cc == 0), stop=(cc == FCC - 1),
                            perf_mode=DR)
                    nc.scalar.activation(z2T[:, ep, :], ps, AF.Identity,
                                         scale=alf_t[:, 0:1],
                                         bias=l2b_t[:, ep:ep + 1])
                    nc.vector.tensor_tensor(z2T[:, ep, :], z2T[:, ep, :],
                                            xT[:, ep, :], ADD)

                out3 = out.ap().rearrange("(ep p) t -> p ep t", p=P)

                def to_out(ko, r):
                    ro = zp.tile([P, TQ], f32, tag="ro")
                    nc.scalar.activation(ro, r, AF.Identity,
                                         scale=n2s_t[:, ko:ko + 1],
                                         bias=n2b_t[:, ko:ko + 1])
                    nc.sync.dma_start(out=out3[:, ko, :], in_=ro)

                layernorm(ctx, z2T, n2s_t, n2b_t, to_out)

        rep_ctx.close()

    nc.compile()
    return nc


def _get_nc():
    global _NC_CACHE
    if _NC_CACHE is None:
        _NC_CACHE = _build_nc()
    return _NC_CACHE


def _pack_dr_lhsT(w, scale=1.0):
    """W [dout, din] -> fp8 DR pack [dout/128, 128(k), din/256 * 2 * 128(m)]:
    pack[mp, k, (cc, j, m)] = W[mp*128+m, cc*256 + j*128 + k]."""
    dout, din = w.shape
    w = (np.asarray(w, np.float32) * scale)
    t = w.reshape(dout // P, P, din // 256, 2, P)       # [mp, m, cc, j, k]
    t = t.transpose(0, 4, 2, 3, 1)                      # [mp, k, cc, j, m]
    return np.ascontiguousarray(t).astype(np_fp8).reshape(
        dout // P, P, (din // 256) * 2 * P)


def _pack_dr_lhsT_pmajor(w, scale=1.0):
    """W [dout, din] -> fp8 DR pack, partition-major resident layout
    [128(k), dout/128 * din/256 * 2 * 128(m)]:
    pack[k, (mp, cc, j, m)] = W[mp*128+m, cc*256 + j*128 + k]."""
    dout, din = w.shape
    w = (np.asarray(w, np.float32) * scale)
    t = w.reshape(dout // P, P, din // 256, 2, P)       # [mp, m, cc, j, k]
    t = t.transpose(4, 0, 2, 3, 1)                      # [k, mp, cc, j, m]
    return np.ascontiguousarray(t).astype(np_fp8).reshape(
        P, (dout // P) * (din // 256) * 2 * P)


def _pack_dr_moving_v_pmajor(w, scale=1.0):
    """v_w [dout, din] -> fp8 DR moving pack, partition-major
    [128(k), din/256 * 2 * dout]: pack[k, (cc, j, n)] = W[n, cc*256+j*128+k]."""
    dout, din = w.shape
    w = (np.asarray(w, np.float32) * scale)
    t = w.reshape(dout, din // 256, 2, P)               # [n, cc, j, k]
    t = t.transpose(3, 1, 2, 0)                         # [k, cc, j, n]
    return np.ascontiguousarray(t).astype(np_fp8).reshape(
        P, (din // 256) * 2 * dout)


def _pack_dr_moving_v(w, scale=1.0):
    """v_w [dout, din] -> fp8 DR moving pack [din/256, 128(k), 2(j)*dout]:
    pack[cc, k, (j, n)] = W[n, cc*256 + j*128 + k]."""
    dout, din = w.shape
    w = (np.asarray(w, np.float32) * scale)
    t = w.reshape(dout, din // 256, 2, P)               # [n, cc, j, k]
    t = t.transpose(1, 3, 2, 0)                         # [cc, k, j, n]
    return np.ascontiguousarray(t).astype(np_fp8).reshape(
        din // 256, P, 2 * dout)


def host_prep(**inputs):
    src = np.asarray(inputs["src"], np.float32)          # [B, T, D]
    attn_mask = np.asarray(inputs["attn_mask"])          # [T, T] bool
    alpha_attn = np.float32(inputs["alpha_attn"])
    alpha_ff = np.float32(inputs["alpha_ff"])

    def col(v, nchunk):
        return np.ascontiguousarray(
            np.asarray(v, np.float32).reshape(nchunk, P).T)

    o_w = np.asarray(inputs["o_w"], np.float32)
    ob_eff = alpha_attn * (np.asarray(inputs["o_b"], np.float32)
                           + o_w @ np.asarray(inputs["v_b"], np.float32))

    shared = {
        "qw": _pack_dr_lhsT(inputs["q_w"]),
        "kw": _pack_dr_lhsT_pmajor(inputs["k_w"]),
        "ow": _pack_dr_lhsT(o_w),
        "vw": _pack_dr_moving_v_pmajor(inputs["v_w"]),
        "l1w": _pack_dr_lhsT(inputs["l1_w"]),
        "l2w": _pack_dr_lhsT(inputs["l2_w"]),
        "qb": col(inputs["q_b"], EC),
        "kb": col(inputs["k_b"], EC),
        "ob": col(ob_eff, EC),
        "l1b": col(inputs["l1_b"], FC),
        "l2b": col(np.asarray(inputs["l2_b"], np.float32) * alpha_ff, EC),
        "al_a": np.full((P, 1), alpha_attn, np.float32),
        "al_f": np.full((P, 1), alpha_ff, np.float32),
        "n1s": col(inputs["n1_s"], EC), "n1b": col(inputs["n1_b"], EC),
        "n2s": col(inputs["n2_s"], EC), "n2b": col(inputs["n2_b"], EC),
    }

    keep = (~attn_mask).astype(np.float32)               # [T(q), T(s)]
    in_maps = []
    for c in range(NCORES):
        b, qq = c // 4, c % 4
        q0 = qq * TQ
        # rotate quarters so the own quarter is processed last (its
        # transposed src stays resident for the residual add)
        order = [x for x in range(NQ) if x != qq] + [qq]
        src_rot = np.ascontiguousarray(
            src[b].reshape(NQ, TQ, D)[order].reshape(T, D)).astype(np_bf16)
        keepT = keep[q0:q0 + TQ, :].T                    # [s, t]
        keepT_rot = keepT.reshape(NQ, TQ, TQ)[order]
        mask_p = np.ascontiguousarray(
            keepT_rot.reshape(NCH, P, TQ).transpose(1, 0, 2)).astype(np_bf16)
        in_maps.append({**shared, "srcb": src_rot, "mask": mask_p})
    return in_maps


def kernel(**inputs):
    in_maps = host_prep(**inputs)
    nc = _get_nc()
    r = run_bass_kernel_spmd(nc, in_maps, core_ids=list(range(NCORES)))

    out = np.empty((B, T, D), np.float32)
    for c in range(NCORES):
        b, qq = c // 4, c % 4
        out[b, qq * TQ:(qq + 1) * TQ, :] = r.results[c]["out"].T
    return out
